# revision 58
# baseline (speedup 1.0000x reference)
"""Windowed multi-head self-attention Bass kernel for Trainium2 (V2).

Shapes (hardcoded): input [64, 256, 1536] fp32 (packed qkv, 32 heads x 16 dim),
rel_bias_table [127, 32] fp32. Output [64, 256, 512] fp32.

Sharding: data-parallel over the window axis B=64 across 8 NeuronCores
(8 windows per core). Bias tables are preprocessed on host and replicated.

V2 structure (vs V1): head-split QS/KD half-tiles (hp 0-7 = "a", 8-15 = "b"),
pair-parity double-buffered (keeps the ~8us stitch-DMA chain off the
critical path).  Scores tiles are [128,512] x 4 PSUM bufs (covers the
PE->exp->PE round-trip latency so the exp engines never slot-starve);
transposes get their own [128,512] x 2 PSUM pool so prep never contends
with scores for banks.  Norm (recip + broadcast-mul) emission is deferred
one half-window so DVE never head-blocks on PE's PV group.  Prep
(transposes/copies/vb) is emitted at the START of each pair so the
PSUM->SBUF copies sit ahead of the exp backlog in the ACT/DVE streams.
Stitch DMAs ride the sync queue (mostly) to keep the ACT sequencer free
for exp dispatch; loads prefetch 2 pairs ahead.  PSUM: 4 (scores) +
2 (pv) + 2 (tr) = 8 banks.
"""

import numpy as np
from contextlib import ExitStack

import concourse.bass as bass
import concourse.bacc as bacc
import concourse.tile as tile
from concourse import mybir
from concourse.bass_utils import run_bass_kernel_spmd

F32 = mybir.dt.float32
BF16 = mybir.dt.bfloat16
U16 = mybir.dt.uint16

NCORES = 8
B = 64
W = B // NCORES
N = 256
C = 1536
NH = 32
HD = 16
SCALE = float(NH) ** -0.5
EXP_A = 128.0 * 1.4426950408889634 * SCALE  # schraudolph mult
EXP_B = 16250.5                             # schraudolph add (calibrated)

NP = W // 2  # window pairs per core

# --- tunables ---
N_EXP_ACT = 39        # of 64 exp instrs per pair on ACT (rest DVE schraudolph)
EXP_N = 64
COPY_PATTERN = "SVSVSVSV"  # 8 tr-copies per pair: S=scalar(ACT), V=vector(DVE)
PV_LAG = 8


def _mk_exp_pattern(n_a, n=EXP_N):
    slots = [None] * n
    for kind, cnt in (("A", n_a), ("D", n - n_a)):
        if cnt == 0:
            continue
        for k in range(cnt):
            i = int(k * n / cnt)
            while slots[i % n] is not None:
                i += 1
            slots[i % n] = kind
    return "".join(slots)


# The 36A/28D spread pattern (verified rel err 1.913e-2) with its 3
# worst-error Schraudolph slots flipped to exact-exp (error can only
# drop per block) -> 39A/25D for better ACT/DVE balance.
EXP_PATTERN = "AAAADADAADADADADAADAAADAADADADADAADADADAADAAADADAADADADAADADADAD"
assert len(EXP_PATTERN) == EXP_N and EXP_PATTERN.count("A") == 39

# engine class per (head, window-in-pair) from the verified slot mapping;
# emission order within a half may then be permuted freely (numerics are
# per-head, order-independent).
EXP_CLS = {}
for _x in range(2):
    for _ws in range(2):
        for _qg in range(2):
            for _u in range(2):
                for _hh in range(4):
                    _slot = (2 * _x + _ws) * 16 + _qg * 8 + _u * 4 + _hh
                    _h = 8 * _x + 4 * _u + 16 * _qg + _hh
                    EXP_CLS[(_h, _ws)] = EXP_PATTERN[_slot]


def _half_order(x, ws):
    # natural emission order (a strict A/D alternation was tried and
    # measured slower -- the scheduler prefers the clustered runs)
    return [(qg, u, hh) for qg in range(2) for u in range(2)
            for hh in range(4)]


def _build_kernel_body(ctx, tc, out, inp, gbias_a, gbias_b, uhot, ident):
    nc = tc.nc

    singles = ctx.enter_context(tc.tile_pool(name="singles", bufs=1))
    inpool = ctx.enter_context(tc.tile_pool(name="inpool", bufs=3))
    tpool = ctx.enter_context(tc.tile_pool(name="tpool", bufs=2))
    vpool = ctx.enter_context(tc.tile_pool(name="vpool", bufs=2))
    ppool = ctx.enter_context(tc.tile_pool(name="ppool", bufs=10))
    opool = ctx.enter_context(tc.tile_pool(name="opool", bufs=3))
    rpool = ctx.enter_context(tc.tile_pool(name="rpool", bufs=4))
    ps_s = ctx.enter_context(tc.tile_pool(name="ps_s", bufs=4, space="PSUM"))
    ps_pv = ctx.enter_context(tc.tile_pool(name="ps_pv", bufs=1, space="PSUM"))
    ps_tr = ctx.enter_context(tc.tile_pool(name="ps_tr", bufs=2, space="PSUM"))

    id_t = singles.tile([128, 128], F32, tag="ident")

    # QS half-tiles [p, j, g, w, n]: head h = 8*x + j + 16*g (x: 0=a, 1=b).
    # rows 0-63: qT bands at 16*(j%4) (dynamic, rest zero); 64-127: G' bias.
    QS = [singles.tile([128, 2, 8, 2, 2, 256], BF16, tag=f"qs{x}", name=f"qs{x}")
          for x in range(2)]
    # KD half-tiles [p, u, qg, w, mch, m]: quad qd = 2*x + u + 4*qg; rows
    # 0-63 kT quad (dynamic), 64-127 one-hot U (static).
    KD = [singles.tile([128, 2, 2, 2, 2, 2, 128], BF16, tag=f"kd{x}",
                       name=f"kd{x}")
          for x in range(2)]

    def emit_init():
        # zero q-regions once (stitches only overwrite the live bands)
        za = QS[0][0:64, :, :, :, :, :]
        zb = QS[1][0:64, :, :, :, :, :]
        nc.gpsimd.memset(za[:, 0], 0.0)
        nc.scalar.memzero(zb[:, 0].rearrange("p j g w n -> p (j g w n)"))

    def emit_init_pr1():
        # DVE is idle at this point in the startup ramp; Pool must stay
        # free for the vb copies pair-0's PV needs.
        za = QS[0][0:64, :, :, :, :, :]
        zb = QS[1][0:64, :, :, :, :, :]
        nc.gpsimd.memset(za[:, 1], 0.0)
        nc.gpsimd.memset(zb[:, 1], 0.0)

    def emit_statics(pr, xs=(0, 1), parts=("u", "g")):
        for x in xs:
            gb = gbias_a if x == 0 else gbias_b
            eng = nc.scalar if (x + pr) % 2 == 0 else nc.sync
            if "u" in parts:
                eng.dma_start(
                    out=KD[x][64:128, pr, :, :, :, :, :].rearrange(
                        "p u q w c m -> p (u q w c m)"),
                    in_=uhot)
            if "g" in parts:
                eng.dma_start(
                    out=QS[x][64:128, pr, :, :, :, :].rearrange(
                        "p j g w n -> p (j g w n)"),
                    in_=gb)

    st = {}

    def emit_load(wp, part=None):
        xin = st.get(wp, {}).get("xin")
        if xin is None:
            xin = {ws: inpool.tile([128, 2, C], F32, tag=f"xin{ws}",
                                   name=f"xin{ws}") for ws in range(2)}
            st[wp] = {"xin": xin}
        lo, hi = {"qk": (0, 1024), "k": (512, 1024), "q": (0, 512),
                  "v": (1024, C)}.get(part, (0, C))
        eng = nc.scalar if part == "v" else nc.sync
        for ws in range(2):
            eng.dma_start(
                out=xin[ws][:, :, lo:hi],
                in_=inp[2 * wp + ws].rearrange(
                    "(c p) f -> p c f", p=128)[:, :, lo:hi])

    trstate = {"ci": 0}

    def _tr_block(wp, sec, dst, cb):
        # transpose 4 [128,128] blocks (2 c-groups x 2 ws) of section
        # sec+cb*128 into a dedicated [128,512] PSUM tile, then one
        # 512-wide copy into dst[:, cb, :, :]
        xin = st[wp]["xin"]
        tr = ps_tr.tile([128, 512], F32, tag="tr", name="tr")
        for ch in range(2):
            trc = tr[:, ch * 256:(ch + 1) * 256]
            for ws in range(2):
                nc.tensor.transpose(
                    trc[:, ws * 128:(ws + 1) * 128],
                    xin[ws][:, ch, sec + cb * 128: sec + (cb + 1) * 128],
                    id_t[:],
                )
        src2 = tr.rearrange("p (c w n) -> p c w n", c=2, w=2)
        eng = (nc.scalar if COPY_PATTERN[trstate["ci"] % len(COPY_PATTERN)]
               == "S" else nc.vector)
        trstate["ci"] += 1
        if eng is nc.scalar:
            nc.scalar.copy(
                dst[:, cb, :, :].rearrange("p w (c n) -> p c w n", c=2), src2)
        else:
            nc.vector.tensor_copy(
                dst[:, cb, :, :].rearrange("p w (c n) -> p c w n", c=2), src2)

    def emit_tr(wp, which, part=None):
        # transposes + PSUM->SBUF bf16 copies for one of q/k (in chunks)
        t = st[wp].get(which)
        if t is None:
            t = tpool.tile([128, 4, 2, 256], BF16, tag=which, name=which)
            st[wp][which] = t
        sec = 0 if which == "qt" else 512
        cbs = {0: (0, 2), 1: (1, 3)}.get(part, (0, 2, 1, 3))
        for cb in cbs:
            _tr_block(wp, sec, t, cb)

    def emit_vb(wp, need_ones):
        xin = st[wp]["xin"]
        vb = {}
        for ws in range(2):
            for ch in range(2):
                t = vpool.tile([128, NH, 17], BF16, tag=f"vb{ws}{ch}",
                               name=f"vb{ws}{ch}")
                nc.gpsimd.tensor_copy(
                    t[:, :, 0:16],
                    xin[ws][:, ch, 1024:1536].rearrange(
                        "p (h d) -> p h d", d=16),
                )
                if need_ones:
                    nc.gpsimd.memset(t[:, :, 16:17], 1.0)
                vb[(ws, ch)] = t
        st[wp]["vb"] = vb

    def emit_stitch(wp, x, si, part=None):
        # stitch half x of pair wp (pr = wp%2); part: "kd"/"qs"/None=both
        pr = wp % 2
        if part in (None, "kd"):
            kt = st[wp]["kt"]
            for u in range(2):
                nc.sync.dma_start(
                    out=KD[x][0:64, pr, u, :, :, :, :].rearrange(
                        "p q w c m -> p q w (c m)"),
                    in_=kt[64 * u:64 * u + 64, x::2, :, :])
        if part in (None, "qs"):
            qt = st[wp]["qt"]
            for j in range(8):
                tq = j % 4
                eng = nc.scalar if j % 4 == 3 else nc.sync
                eng.dma_start(
                    out=QS[x][16 * tq:16 * tq + 16, pr, j, :, :, :],
                    in_=qt[16 * j:16 * j + 16, x::2, :, :])

    def emit_norm(ws, x, pvn, oacc):
        for nch in range(2):
            pvv = pvn[nch].rearrange("p (h s) -> p h s", s=17)
            rcp = rpool.tile([128, 16, 1], F32, tag=f"rcp{nch}",
                             name=f"rcp{nch}")
            nc.vector.reciprocal(rcp[:], pvv[:, :, 16:17])
            rb = rcp[:]
            hstride = rb.ap[1][0]
            rbcast = bass.AP(
                tensor=rb.tensor, offset=rb.offset,
                ap=[rb.ap[0], [8 * hstride, 2], [hstride, 8], [0, 16]],
            )
            # out cols per head h = 16h; half x heads at (g, j): col
            # 256g + 128x + 16j -> 4D strided view [p, g, j, d]
            oc = oacc[:, nch, 128 * x:]
            oview = bass.AP(
                tensor=oc.tensor, offset=oc.offset,
                ap=[oc.ap[0], [256 * oc.ap[-1][0], 2],
                    [16 * oc.ap[-1][0], 8], [oc.ap[-1][0], 16]],
            )
            nc.vector.tensor_mul(
                oview,
                pvv.rearrange("p (g j) s -> p g j s", g=2)[:, :, :, 0:16],
                rbcast,
            )

    expstate = {"i": 0}
    normq = []  # deferred norm emissions (one half-window of lag)

    def flush_norm():
        while normq:
            normq.pop(0)()

    def emit_half(wp, x, ws, hooks=()):
        # scores + exp + PV for heads {8x..8x+7} u {16+8x..16+8x+7} of
        # window ws; norm is queued and emitted early in the NEXT half so
        # DVE never head-blocks waiting for PE's PV group.
        # hooks: list of (after_tile_idx, fn) emission callbacks
        vb = st[wp]["vb"]
        oacc = st[wp].setdefault("oacc", {})
        if ws not in oacc:
            oacc[ws] = opool.tile([128, 2, 512], F32, tag="oacc", name="oacc")
        oa = oacc[ws]
        pvn = [ps_pv.tile([128, 272], F32, tag=f"pv{nch}", name=f"pv{nch}")
               for nch in range(2)]

        def emit_pv(pt, h):
            jj = (h % 8) + 8 * (h // 16)
            for nch in range(2):
                for mch in range(2):
                    nc.tensor.matmul(
                        pvn[nch][:, 17 * jj:17 * jj + 17],
                        lhsT=pt[:, mch * 256 + nch * 128:
                                mch * 256 + nch * 128 + 128],
                        rhs=vb[(ws, mch)][:, h, :],
                        start=(mch == 0),
                        stop=(mch == 1),
                    )

        pending = []
        hooks = dict(hooks)
        ti = 0
        for qg, u, hh in _half_order(x, ws):
            qd = 2 * x + u + 4 * qg
            h = 4 * qd + hh
            j, g = (h % 16) % 8, h // 16
            ps = ps_s.tile([128, 512], F32, tag="scores",
                           name="scores")
            for mch in range(2):
                nc.tensor.matmul(
                    ps[:, mch * 256:(mch + 1) * 256],
                    lhsT=KD[x][:, wp % 2, u, qg, ws, mch, :],
                    rhs=QS[x][:, wp % 2, j, g, ws, :],
                    start=True,
                    stop=True,
                )
            pt = ppool.tile([128, 512], BF16, tag="pt", name="pt")
            kind = EXP_CLS[(h, ws)]
            if kind == "A":
                nc.scalar.activation(
                    pt[:], ps[:], mybir.ActivationFunctionType.Exp,
                    scale=SCALE,
                )
            else:
                nc.vector.tensor_scalar(
                    pt[:].bitcast(U16), ps[:], EXP_A, EXP_B,
                    mybir.AluOpType.mult, mybir.AluOpType.add,
                )
            pending.append((pt, h))
            if len(pending) > PV_LAG:
                emit_pv(*pending.pop(0))
            if ti == 2:
                flush_norm()
            if ti in hooks:
                hooks[ti]()
            ti += 1
        while pending:
            emit_pv(*pending.pop(0))

        def _norm_and_store():
            emit_norm(ws, x, pvn, oacc=oa)
            if x == 1:  # oacc complete after half b's norm
                emit_store(wp, ws)
        normq.append(_norm_and_store)

    def emit_store(wp, ws):
        oa = st[wp]["oacc"][ws]
        nc.sync.dma_start(
            out=out[2 * wp + ws].rearrange("(c p) n -> p c n", p=128),
            in_=oa[:])

    # ---- prologue ----
    nc.sync.dma_start(out=id_t[:], in_=ident)
    emit_load(0, "k")
    emit_load(0, "q")
    # PE p-state warmup: dummy bf16 transposes keep the tensor engine
    # continuously busy from ~0.3us so the real transposes (arriving when
    # load(0,k) lands ~4us) run at the full 2.4 GHz p-state.
    warm = singles.tile([128, 128], BF16, tag="warm")
    nc.gpsimd.memset(warm[:], 1.0)
    for wi in range(9):
        wt = ps_tr.tile([128, 512], F32, tag="tr", name="tr")
        for wk in range(4):
            nc.tensor.transpose(
                wt[:, 64 * wk:64 * wk + 64].bitcast(BF16),
                warm[:], warm[:])
    emit_init()
    emit_statics(0, xs=(0,))            # uhot_a + gbias_a (scalar queue)
    emit_tr(0, "kt")
    emit_tr(0, "qt")
    emit_stitch(0, 0, 0, "kd")
    emit_stitch(0, 0, 0, "qs")          # half-a ready first
    emit_load(0, "v")
    emit_statics(0, xs=(1,), parts="u")
    emit_stitch(0, 1, 1, "kd")
    emit_statics(0, xs=(1,), parts="g")
    emit_stitch(0, 1, 1, "qs")
    emit_vb(0, need_ones=True)
    emit_load(1)

    # ---- steady-state pair loop ----
    def _noop():
        pass

    for wp in range(NP):
        nxt = wp + 1 < NP
        # half a (x=0): also prep next pair (PE transposes into own PSUM
        # pool + copies interleave in the exp streams; xin ready long ago)
        ha0 = ((0, (lambda: emit_tr(wp + 1, "kt", 0)) if nxt else _noop),
               (1, (lambda: emit_init_pr1()) if wp == 0 else _noop),
               (2, (lambda: emit_tr(wp + 1, "kt", 1)) if nxt else _noop),
               (4, (lambda: emit_tr(wp + 1, "qt", 0)) if nxt else _noop),
               (6, (lambda: emit_tr(wp + 1, "qt", 1)) if nxt else _noop),
               (8, lambda: (emit_load(wp + 2) if wp + 2 < NP else None)),
               (10, (lambda: emit_vb(wp + 1, need_ones=(wp + 1 < 2)))
                if nxt else _noop))
        emit_half(wp, 0, 0, hooks=ha0)
        emit_half(wp, 0, 1)
        # half b (x=1); stitch half-a of next pair once QS[0]/KD[0] free
        hooks_b0 = [(3, lambda: emit_stitch(wp + 1, 0, wp % 2))] if nxt \
            else []
        if wp == 0:
            hooks_b0.append((8, lambda: emit_statics(1)))
        emit_half(wp, 1, 0, hooks=tuple(hooks_b0))
        emit_half(wp, 1, 1)
        if nxt:
            emit_stitch(wp + 1, 1, (wp + 1) % 2)
        st.pop(wp - 1, None)
    flush_norm()


def build_nc():
    nc = bacc.Bacc(
        "TRN2", target_bir_lowering=False, debug=False, num_devices=NCORES
    )
    inp = nc.dram_tensor("inp", [W, N, C], F32, kind="ExternalInput").ap()
    gbias_a = nc.dram_tensor("gbias_a", [64, 8192], BF16,
                             kind="ExternalInput").ap()
    gbias_b = nc.dram_tensor("gbias_b", [64, 8192], BF16,
                             kind="ExternalInput").ap()
    uhot = nc.dram_tensor("uhot", [64, 2048], BF16,
                          kind="ExternalInput").ap()
    ident = nc.dram_tensor("ident", [128, 128], F32, kind="ExternalInput").ap()
    out = nc.dram_tensor("out", [W, N, NH * HD], F32,
                         kind="ExternalOutput").ap()
    with tile.TileContext(nc) as tc:
        with ExitStack() as ctx:
            _build_kernel_body(ctx, tc, out, inp, gbias_a, gbias_b, uhot,
                               ident)
    nc.compile()
    return nc


def _host_consts(table):
    import ml_dtypes
    bf16 = ml_dtypes.bfloat16
    # G'[i, h, n] = table[n//4 - i + 63, h]/SCALE
    j = np.arange(N) // 4
    i0 = np.arange(64)
    idx = j[None, :] - i0[:, None] + 63  # [64, 256]
    g = table[idx]  # [64, 256, NH]
    gb = np.transpose(g, (2, 0, 1)) * np.float32(1.0 / SCALE)  # [NH, 64, 256]
    gbias = np.empty((2, 64, 8, 2, 2, 256), dtype=np.float32)
    for h in range(NH):
        x, jj, gg = (h % 16) // 8, (h % 16) % 8, h // 16
        gbias[x, :, jj, gg, 0, :] = gb[h]
        gbias[x, :, jj, gg, 1, :] = gb[h]
    gbias = gbias.reshape(2, 64, 8192)
    # U[i, u, qg, w, mch, m] = 1 if (m//4 + 32*mch) == i
    m4 = np.arange(128) // 4
    u = (m4[None, None, :] + 32 * np.arange(2)[None, :, None]
         == np.arange(64)[:, None, None]).astype(np.float32)  # [64, 2, 128]
    uhot = np.broadcast_to(
        u[:, None, None, None, :, :], (64, 2, 2, 2, 2, 128)).reshape(64, 2048)
    ident = np.eye(128, dtype=np.float32)
    return (np.ascontiguousarray(gbias[0].astype(bf16)),
            np.ascontiguousarray(gbias[1].astype(bf16)),
            np.ascontiguousarray(uhot.astype(bf16)), ident)


_NC_CACHE = None


def kernel(input, rel_bias_table):
    global _NC_CACHE
    x = np.ascontiguousarray(np.asarray(input, dtype=np.float32))
    tbl = np.asarray(rel_bias_table, dtype=np.float32)
    assert x.shape == (B, N, C), x.shape
    assert tbl.shape == (127, NH), tbl.shape

    if _NC_CACHE is None:
        _NC_CACHE = build_nc()
    nc = _NC_CACHE

    gbias_a, gbias_b, uhot, ident = _host_consts(tbl)
    in_maps = [
        {
            "inp": np.ascontiguousarray(x[i * W:(i + 1) * W]),
            "gbias_a": gbias_a,
            "gbias_b": gbias_b,
            "uhot": uhot,
            "ident": ident,
        }
        for i in range(NCORES)
    ]
    res = run_bass_kernel_spmd(nc, in_maps, list(range(NCORES)))
    return np.concatenate([res.results[i]["out"] for i in range(NCORES)],
                          axis=0)


# revision 69
# speedup vs baseline: 1.0152x; 1.0152x over previous
"""Windowed multi-head self-attention Bass kernel for Trainium2 (V2).

Shapes (hardcoded): input [64, 256, 1536] fp32 (packed qkv, 32 heads x 16 dim),
rel_bias_table [127, 32] fp32. Output [64, 256, 512] fp32.

Sharding: data-parallel over the window axis B=64 across 8 NeuronCores
(8 windows per core). Bias tables are preprocessed on host and replicated.

V2 structure (vs V1): head-split QS/KD half-tiles (hp 0-7 = "a", 8-15 = "b"),
pair-parity double-buffered (keeps the ~8us stitch-DMA chain off the
critical path).  Scores tiles are [128,512] x 4 PSUM bufs (covers the
PE->exp->PE round-trip latency so the exp engines never slot-starve);
transposes get their own [128,512] x 2 PSUM pool so prep never contends
with scores for banks.  Norm (recip + broadcast-mul) emission is deferred
one half-window so DVE never head-blocks on PE's PV group.  Prep
(transposes/copies/vb) is emitted at the START of each pair so the
PSUM->SBUF copies sit ahead of the exp backlog in the ACT/DVE streams.
Stitch DMAs ride the sync queue (mostly) to keep the ACT sequencer free
for exp dispatch; loads prefetch 2 pairs ahead.  PSUM: 4 (scores) +
2 (pv) + 2 (tr) = 8 banks.
"""

import numpy as np
from contextlib import ExitStack

import concourse.bass as bass
import concourse.bacc as bacc
import concourse.tile as tile
from concourse import mybir
from concourse.bass_utils import run_bass_kernel_spmd

F32 = mybir.dt.float32
BF16 = mybir.dt.bfloat16
U16 = mybir.dt.uint16

NCORES = 8
B = 64
W = B // NCORES
N = 256
C = 1536
NH = 32
HD = 16
SCALE = float(NH) ** -0.5
EXP_A = 128.0 * 1.4426950408889634 * SCALE  # schraudolph mult
EXP_B = 16250.5                             # schraudolph add (calibrated)

NP = W // 2  # window pairs per core

# --- tunables ---
N_EXP_ACT = 39        # of 64 exp instrs per pair on ACT (rest DVE schraudolph)
EXP_N = 64
COPY_PATTERN = "SVSVSVSV"  # 8 tr-copies per pair: S=scalar(ACT), V=vector(DVE)
PV_LAG = 6


def _mk_exp_pattern(n_a, n=EXP_N):
    slots = [None] * n
    for kind, cnt in (("A", n_a), ("D", n - n_a)):
        if cnt == 0:
            continue
        for k in range(cnt):
            i = int(k * n / cnt)
            while slots[i % n] is not None:
                i += 1
            slots[i % n] = kind
    return "".join(slots)


# The 36A/28D spread pattern (verified rel err 1.913e-2) with its 3
# worst-error Schraudolph slots flipped to exact-exp (error can only
# drop per block) -> 39A/25D for better ACT/DVE balance.
EXP_PATTERN = "AAAADADAADADADADAADAAADAADADADADAADADADAADAAADADAADADADAADADADAD"
assert len(EXP_PATTERN) == EXP_N and EXP_PATTERN.count("A") == 39

# engine class per (head, window-in-pair) from the verified slot mapping;
# emission order within a half may then be permuted freely (numerics are
# per-head, order-independent).
EXP_CLS = {}
for _x in range(2):
    for _ws in range(2):
        for _qg in range(2):
            for _u in range(2):
                for _hh in range(4):
                    _slot = (2 * _x + _ws) * 16 + _qg * 8 + _u * 4 + _hh
                    _h = 8 * _x + 4 * _u + 16 * _qg + _hh
                    EXP_CLS[(_h, _ws)] = EXP_PATTERN[_slot]


def _half_order(x, ws):
    # natural emission order (a strict A/D alternation was tried and
    # measured slower -- the scheduler prefers the clustered runs)
    return [(qg, u, hh) for qg in range(2) for u in range(2)
            for hh in range(4)]


def _build_kernel_body(ctx, tc, out, inp, gbias_a, gbias_b, uhot, ident):
    nc = tc.nc

    singles = ctx.enter_context(tc.tile_pool(name="singles", bufs=1))
    inpool = ctx.enter_context(tc.tile_pool(name="inpool", bufs=3))
    tpool = ctx.enter_context(tc.tile_pool(name="tpool", bufs=2))
    vpool = ctx.enter_context(tc.tile_pool(name="vpool", bufs=2))
    ppool = ctx.enter_context(tc.tile_pool(name="ppool", bufs=12))
    opool = ctx.enter_context(tc.tile_pool(name="opool", bufs=3))
    rpool = ctx.enter_context(tc.tile_pool(name="rpool", bufs=4))
    ps_s = ctx.enter_context(tc.tile_pool(name="ps_s", bufs=4, space="PSUM"))
    ps_pv = ctx.enter_context(tc.tile_pool(name="ps_pv", bufs=1, space="PSUM"))
    ps_tr = ctx.enter_context(tc.tile_pool(name="ps_tr", bufs=2, space="PSUM"))

    id_t = singles.tile([128, 128], F32, tag="ident")

    # QS half-tiles [p, j, g, w, n]: head h = 8*x + j + 16*g (x: 0=a, 1=b).
    # rows 0-63: qT bands at 16*(j%4) (dynamic, rest zero); 64-127: G' bias.
    QS = [singles.tile([128, 2, 8, 2, 2, 256], BF16, tag=f"qs{x}", name=f"qs{x}")
          for x in range(2)]
    # KD half-tiles [p, u, qg, w, mch, m]: quad qd = 2*x + u + 4*qg; rows
    # 0-63 kT quad (dynamic), 64-127 one-hot U (static).
    KD = [singles.tile([128, 2, 2, 2, 2, 2, 128], BF16, tag=f"kd{x}",
                       name=f"kd{x}")
          for x in range(2)]

    def emit_init():
        # zero q-regions once (stitches only overwrite the live bands)
        za = QS[0][0:64, :, :, :, :, :]
        zb = QS[1][0:64, :, :, :, :, :]
        nc.gpsimd.memset(za[:, 0], 0.0)
        nc.scalar.memzero(zb[:, 0].rearrange("p j g w n -> p (j g w n)"))

    def emit_init_pr1():
        # DVE is idle at this point in the startup ramp; Pool must stay
        # free for the vb copies pair-0's PV needs.
        za = QS[0][0:64, :, :, :, :, :]
        zb = QS[1][0:64, :, :, :, :, :]
        nc.gpsimd.memset(za[:, 1], 0.0)
        nc.gpsimd.memset(zb[:, 1], 0.0)

    def emit_statics(pr, xs=(0, 1), parts=("u", "g")):
        for x in xs:
            gb = gbias_a if x == 0 else gbias_b
            eng = nc.scalar if (x + pr) % 2 == 0 else nc.sync
            if "u" in parts:
                eng.dma_start(
                    out=KD[x][64:128, pr, :, :, :, :, :].rearrange(
                        "p u q w c m -> p (u q w c m)"),
                    in_=uhot)
            if "g" in parts:
                eng.dma_start(
                    out=QS[x][64:128, pr, :, :, :, :].rearrange(
                        "p j g w n -> p (j g w n)"),
                    in_=gb)

    def emit_statics_pr1_copy():
        # replicate pr-0 static rows to pr-1 with DVE bf16 SBUF copies
        # (4x mode) during DVE's idle startup window -- saves 7.3us of
        # DMA on the startup-critical DMA_ENGINES device.
        for x in range(2):
            nc.vector.tensor_copy(
                KD[x][64:128, 1, :, :, :, :, :],
                KD[x][64:128, 0, :, :, :, :, :])
            nc.vector.tensor_copy(
                QS[x][64:128, 1, :, :, :, :],
                QS[x][64:128, 0, :, :, :, :])

    st = {}

    def emit_load(wp, part=None):
        xin = st.get(wp, {}).get("xin")
        if xin is None:
            xin = {ws: inpool.tile([128, 2, C], F32, tag=f"xin{ws}",
                                   name=f"xin{ws}") for ws in range(2)}
            st[wp] = {"xin": xin}
        lo, hi = {"qk": (0, 1024), "k": (512, 1024), "q": (0, 512),
                  "v": (1024, C)}.get(part, (0, C))
        eng = nc.scalar if part == "v" else nc.sync
        for ws in range(2):
            eng.dma_start(
                out=xin[ws][:, :, lo:hi],
                in_=inp[2 * wp + ws].rearrange(
                    "(c p) f -> p c f", p=128)[:, :, lo:hi])

    trstate = {"ci": 0}

    def _tr_block(wp, sec, dst, cb):
        # transpose 4 [128,128] blocks (2 c-groups x 2 ws) of section
        # sec+cb*128 into a dedicated [128,512] PSUM tile, then one
        # 512-wide copy into dst[:, cb, :, :]
        xin = st[wp]["xin"]
        tr = ps_tr.tile([128, 512], F32, tag="tr", name="tr")
        for ch in range(2):
            trc = tr[:, ch * 256:(ch + 1) * 256]
            for ws in range(2):
                nc.tensor.transpose(
                    trc[:, ws * 128:(ws + 1) * 128],
                    xin[ws][:, ch, sec + cb * 128: sec + (cb + 1) * 128],
                    id_t[:],
                )
        src2 = tr.rearrange("p (c w n) -> p c w n", c=2, w=2)
        eng = (nc.scalar if COPY_PATTERN[trstate["ci"] % len(COPY_PATTERN)]
               == "S" else nc.vector)
        trstate["ci"] += 1
        if eng is nc.scalar:
            nc.scalar.copy(
                dst[:, cb, :, :].rearrange("p w (c n) -> p c w n", c=2), src2)
        else:
            nc.vector.tensor_copy(
                dst[:, cb, :, :].rearrange("p w (c n) -> p c w n", c=2), src2)

    def emit_tr(wp, which, part=None):
        # transposes + PSUM->SBUF bf16 copies for one of q/k (in chunks)
        t = st[wp].get(which)
        if t is None:
            t = tpool.tile([128, 4, 2, 256], BF16, tag=which, name=which)
            st[wp][which] = t
        sec = 0 if which == "qt" else 512
        cbs = {0: (0, 2), 1: (1, 3)}.get(part, (0, 2, 1, 3))
        for cb in cbs:
            _tr_block(wp, sec, t, cb)

    def emit_vb(wp, need_ones):
        xin = st[wp]["xin"]
        vb = {}
        for ws in range(2):
            for ch in range(2):
                t = vpool.tile([128, NH, 17], BF16, tag=f"vb{ws}{ch}",
                               name=f"vb{ws}{ch}")
                nc.gpsimd.tensor_copy(
                    t[:, :, 0:16],
                    xin[ws][:, ch, 1024:1536].rearrange(
                        "p (h d) -> p h d", d=16),
                )
                if need_ones:
                    nc.gpsimd.memset(t[:, :, 16:17], 1.0)
                vb[(ws, ch)] = t
        st[wp]["vb"] = vb

    def emit_stitch(wp, x, si, part=None):
        # stitch half x of pair wp (pr = wp%2); part: "kd"/"qs"/None=both
        pr = wp % 2
        if part in (None, "kd"):
            kt = st[wp]["kt"]
            for u in range(2):
                nc.sync.dma_start(
                    out=KD[x][0:64, pr, u, :, :, :, :].rearrange(
                        "p q w c m -> p q w (c m)"),
                    in_=kt[64 * u:64 * u + 64, x::2, :, :])
        if part in (None, "qs"):
            qt = st[wp]["qt"]
            for j in range(8):
                tq = j % 4
                eng = nc.scalar if j % 4 == 3 else nc.sync
                eng.dma_start(
                    out=QS[x][16 * tq:16 * tq + 16, pr, j, :, :, :],
                    in_=qt[16 * j:16 * j + 16, x::2, :, :])

    def emit_norm(ws, x, pvn, oacc):
        for nch in range(2):
            pvv = pvn[nch].rearrange("p (h s) -> p h s", s=17)
            rcp = rpool.tile([128, 16, 1], F32, tag=f"rcp{nch}",
                             name=f"rcp{nch}")
            nc.vector.reciprocal(rcp[:], pvv[:, :, 16:17])
            rb = rcp[:]
            hstride = rb.ap[1][0]
            rbcast = bass.AP(
                tensor=rb.tensor, offset=rb.offset,
                ap=[rb.ap[0], [8 * hstride, 2], [hstride, 8], [0, 16]],
            )
            # out cols per head h = 16h; half x heads at (g, j): col
            # 256g + 128x + 16j -> 4D strided view [p, g, j, d]
            oc = oacc[:, nch, 128 * x:]
            oview = bass.AP(
                tensor=oc.tensor, offset=oc.offset,
                ap=[oc.ap[0], [256 * oc.ap[-1][0], 2],
                    [16 * oc.ap[-1][0], 8], [oc.ap[-1][0], 16]],
            )
            nc.vector.tensor_mul(
                oview,
                pvv.rearrange("p (g j) s -> p g j s", g=2)[:, :, :, 0:16],
                rbcast,
            )

    expstate = {"i": 0}
    normq = []  # deferred norm emissions (one half-window of lag)

    def flush_norm():
        while normq:
            normq.pop(0)()

    def emit_half(wp, x, ws, hooks=(), lag=PV_LAG):
        # scores + exp + PV for heads {8x..8x+7} u {16+8x..16+8x+7} of
        # window ws; norm is queued and emitted early in the NEXT half so
        # DVE never head-blocks waiting for PE's PV group.
        # hooks: list of (after_tile_idx, fn) emission callbacks
        vb = st[wp]["vb"]
        oacc = st[wp].setdefault("oacc", {})
        if ws not in oacc:
            oacc[ws] = opool.tile([128, 2, 512], F32, tag="oacc", name="oacc")
        oa = oacc[ws]
        pvn = [ps_pv.tile([128, 272], F32, tag=f"pv{nch}", name=f"pv{nch}")
               for nch in range(2)]

        def emit_pv(pt, h):
            jj = (h % 8) + 8 * (h // 16)
            for nch in range(2):
                for mch in range(2):
                    nc.tensor.matmul(
                        pvn[nch][:, 17 * jj:17 * jj + 17],
                        lhsT=pt[:, mch * 256 + nch * 128:
                                mch * 256 + nch * 128 + 128],
                        rhs=vb[(ws, mch)][:, h, :],
                        start=(mch == 0),
                        stop=(mch == 1),
                    )

        pending = []
        hooks = dict(hooks)
        ti = 0
        for qg, u, hh in _half_order(x, ws):
            qd = 2 * x + u + 4 * qg
            h = 4 * qd + hh
            j, g = (h % 16) % 8, h // 16
            ps = ps_s.tile([128, 512], F32, tag="scores",
                           name="scores")
            for mch in range(2):
                nc.tensor.matmul(
                    ps[:, mch * 256:(mch + 1) * 256],
                    lhsT=KD[x][:, wp % 2, u, qg, ws, mch, :],
                    rhs=QS[x][:, wp % 2, j, g, ws, :],
                    start=True,
                    stop=True,
                )
            pt = ppool.tile([128, 512], BF16, tag="pt", name="pt")
            kind = EXP_CLS[(h, ws)]
            if kind == "A":
                nc.scalar.activation(
                    pt[:], ps[:], mybir.ActivationFunctionType.Exp,
                    scale=SCALE,
                )
            else:
                nc.vector.tensor_scalar(
                    pt[:].bitcast(U16), ps[:], EXP_A, EXP_B,
                    mybir.AluOpType.mult, mybir.AluOpType.add,
                )
            pending.append((pt, h))
            if len(pending) > lag:
                emit_pv(*pending.pop(0))
            if ti == 2:
                flush_norm()
            if ti in hooks:
                hooks[ti]()
            ti += 1
        while pending:
            emit_pv(*pending.pop(0))

        def _norm_and_store():
            emit_norm(ws, x, pvn, oacc=oa)
            if x == 1:  # oacc complete after half b's norm
                emit_store(wp, ws)
        normq.append(_norm_and_store)

    def emit_store(wp, ws):
        oa = st[wp]["oacc"][ws]
        nc.sync.dma_start(
            out=out[2 * wp + ws].rearrange("(c p) n -> p c n", p=128),
            in_=oa[:])

    # ---- prologue ----
    nc.sync.dma_start(out=id_t[:], in_=ident)
    emit_load(0, "k")
    emit_load(0, "q")
    # PE p-state warmup: dummy bf16 transposes keep the tensor engine
    # continuously busy from ~0.3us so the real transposes (arriving when
    # load(0,k) lands ~4us) run at the full 2.4 GHz p-state.
    warm = singles.tile([128, 128], BF16, tag="warm")
    nc.gpsimd.memset(warm[:], 1.0)
    for wi in range(9):
        wt = ps_tr.tile([128, 512], F32, tag="tr", name="tr")
        for wk in range(4):
            nc.tensor.transpose(
                wt[:, 64 * wk:64 * wk + 64].bitcast(BF16),
                warm[:], warm[:])
    emit_init()
    emit_statics(0, xs=(0,))            # uhot_a + gbias_a (scalar queue)
    emit_tr(0, "kt")
    emit_tr(0, "qt")
    emit_stitch(0, 0, 0, "kd")
    emit_stitch(0, 0, 0, "qs")          # half-a ready first
    emit_statics(0, xs=(1,), parts="u")
    emit_stitch(0, 1, 1, "kd")
    emit_statics(0, xs=(1,), parts="g")
    emit_load(0, "v")
    emit_stitch(0, 1, 1, "qs")
    emit_statics_pr1_copy()
    emit_vb(0, need_ones=True)
    emit_load(1)

    # ---- steady-state pair loop ----
    def _noop():
        pass

    for wp in range(NP):
        nxt = wp + 1 < NP
        # half a (x=0): also prep next pair (PE transposes into own PSUM
        # pool + copies interleave in the exp streams; xin ready long ago)
        ha0 = ((0, (lambda: emit_tr(wp + 1, "kt", 0)) if nxt else _noop),
               (1, (lambda: emit_init_pr1()) if wp == 0 else _noop),
               (2, (lambda: emit_tr(wp + 1, "kt", 1)) if nxt else _noop),
               (4, (lambda: emit_tr(wp + 1, "qt", 0)) if nxt else _noop),
               (6, (lambda: emit_tr(wp + 1, "qt", 1)) if nxt else _noop),
               (8, lambda: (emit_load(wp + 2) if wp + 2 < NP else None)),
               (10, (lambda: emit_vb(wp + 1, need_ones=(wp + 1 < 2)))
                if nxt else _noop))
        emit_half(wp, 0, 0, hooks=ha0)
        emit_half(wp, 0, 1)
        # half b (x=1); stitch half-a of next pair once QS[0]/KD[0] free
        hooks_b0 = [(3, lambda: emit_stitch(wp + 1, 0, wp % 2))] if nxt \
            else []
        emit_half(wp, 1, 0, hooks=tuple(hooks_b0))
        emit_half(wp, 1, 1)
        if nxt:
            emit_stitch(wp + 1, 1, (wp + 1) % 2)
        st.pop(wp - 1, None)
    flush_norm()


def build_nc():
    nc = bacc.Bacc(
        "TRN2", target_bir_lowering=False, debug=False, num_devices=NCORES
    )
    inp = nc.dram_tensor("inp", [W, N, C], F32, kind="ExternalInput").ap()
    gbias_a = nc.dram_tensor("gbias_a", [64, 8192], BF16,
                             kind="ExternalInput").ap()
    gbias_b = nc.dram_tensor("gbias_b", [64, 8192], BF16,
                             kind="ExternalInput").ap()
    uhot = nc.dram_tensor("uhot", [64, 2048], BF16,
                          kind="ExternalInput").ap()
    ident = nc.dram_tensor("ident", [128, 128], F32, kind="ExternalInput").ap()
    out = nc.dram_tensor("out", [W, N, NH * HD], F32,
                         kind="ExternalOutput").ap()
    with tile.TileContext(nc) as tc:
        with ExitStack() as ctx:
            _build_kernel_body(ctx, tc, out, inp, gbias_a, gbias_b, uhot,
                               ident)
    nc.compile()
    return nc


def _host_consts(table):
    import ml_dtypes
    bf16 = ml_dtypes.bfloat16
    # G'[i, h, n] = table[n//4 - i + 63, h]/SCALE
    j = np.arange(N) // 4
    i0 = np.arange(64)
    idx = j[None, :] - i0[:, None] + 63  # [64, 256]
    g = table[idx]  # [64, 256, NH]
    gb = np.transpose(g, (2, 0, 1)) * np.float32(1.0 / SCALE)  # [NH, 64, 256]
    gbias = np.empty((2, 64, 8, 2, 2, 256), dtype=np.float32)
    for h in range(NH):
        x, jj, gg = (h % 16) // 8, (h % 16) % 8, h // 16
        gbias[x, :, jj, gg, 0, :] = gb[h]
        gbias[x, :, jj, gg, 1, :] = gb[h]
    gbias = gbias.reshape(2, 64, 8192)
    # U[i, u, qg, w, mch, m] = 1 if (m//4 + 32*mch) == i
    m4 = np.arange(128) // 4
    u = (m4[None, None, :] + 32 * np.arange(2)[None, :, None]
         == np.arange(64)[:, None, None]).astype(np.float32)  # [64, 2, 128]
    uhot = np.broadcast_to(
        u[:, None, None, None, :, :], (64, 2, 2, 2, 2, 128)).reshape(64, 2048)
    ident = np.eye(128, dtype=np.float32)
    return (np.ascontiguousarray(gbias[0].astype(bf16)),
            np.ascontiguousarray(gbias[1].astype(bf16)),
            np.ascontiguousarray(uhot.astype(bf16)), ident)


_NC_CACHE = None


def kernel(input, rel_bias_table):
    global _NC_CACHE
    x = np.ascontiguousarray(np.asarray(input, dtype=np.float32))
    tbl = np.asarray(rel_bias_table, dtype=np.float32)
    assert x.shape == (B, N, C), x.shape
    assert tbl.shape == (127, NH), tbl.shape

    if _NC_CACHE is None:
        _NC_CACHE = build_nc()
    nc = _NC_CACHE

    gbias_a, gbias_b, uhot, ident = _host_consts(tbl)
    in_maps = [
        {
            "inp": np.ascontiguousarray(x[i * W:(i + 1) * W]),
            "gbias_a": gbias_a,
            "gbias_b": gbias_b,
            "uhot": uhot,
            "ident": ident,
        }
        for i in range(NCORES)
    ]
    res = run_bass_kernel_spmd(nc, in_maps, list(range(NCORES)))
    return np.concatenate([res.results[i]["out"] for i in range(NCORES)],
                          axis=0)
